# revision 59
# baseline (speedup 1.0000x reference)
"""Multi-head causal attention (B=4, N=2048, D=1024, H=16, d=64) on 8 TRN2 cores.

Sharding: core c handles batch b = c//2 and head-group hg = c%2 (8 heads).
Each core computes Q/K/V projections for its heads, causal flash-style
attention, and a partial output projection; the host sums the two partials
per batch (all-reduce done host-side) and transposes back.

All four projections run as 3-term fp8e4m3 hi/lo DoubleRow matmuls at 0.75x
the bf16 PE cost with better-than-bf16 accuracy: operands are split into
fp8 hi + unscaled fp8 lo residual (x/W splits on the host for free; the
attention output's split on the DVE), and each psum accumulates
  hi.hi (two 128-chunks per DoubleRow instr) + (lo.hi + hi.lo) (the two
  cross terms as the two k-tiles of one DoubleRow instr), dropping lo.lo.
W's are pre-scaled by WS=64 (descale folded into the exp scale / the final
copy); the attention-output split is kept at 16*o_norm via an extra ln(4)
in the reciprocal so extremes stay inside e4m3 range. Raw fp8 (un-split)
fails the 2e-2 budget everywhere (proj 6.6e-2, QK 4.5e-2, PV 3.1e-2 even
renormalized), so QK/PV stay bf16 -- both are at their structural PE floor
(QK output-bound at M=128, PV contraction-bound at 128 keys).

Attention per (q-chunk, head-pair), transposed layouts throughout:
  S^T = K Q^T per 128-key block (lhsT=KT, K=64, 2 heads side by side in F)
  P^T = exp(S^T * SCALE/WS^2) on ACT straight from PSUM
  rowsums via ones-augmented V (V'' = [V | 1*64], M=128) during the PV
        matmul -- PSUM rows 64:127 hold the rowsum replicated across
        partitions, so no partition-broadcast is ever needed
  0.25/rowsum = exp(-ln(4*rowsum)) on ACT, deferred 4 chunks so it hides
        inside later exp streams.

Schedule: the exp stream (1 elem/partition/cycle on ACT) is ~22% slower
than the QK+PV PE work per key block, so projections are queued as
single-instruction steps and fed into the kb loop between the QK prefetch
and the PV, keeping the PE busy while exp cooks ("pend_hard" = next
stream's projections, deadline-spread; "pend_soft" = out-projections,
fed when slack). Loops run qc-outer so each chunk's out-projection (and
its 2MB output DMA) overlaps the next chunk's attention. Input DMAs are
contiguous-per-partition groups (128 descriptors each), priority-ordered
so the 1.5MB needed by qk_proj(0,0) lands first.

Pitfalls encoded here: GPSIMD cannot touch PSUM (all psum drains go via
nc.any to ACT/DVE); DVE has no float divide; fp8 stores do not saturate
(overflow => inf => NaN); multi-wait instructions are re-legalized into
single-wait NoOp chains for this container's walrus.
"""

import sys

import numpy as np

if "/opt/trn_rl_repo" not in sys.path:
    sys.path.insert(0, "/opt/trn_rl_repo")

import ml_dtypes

B, N, D, H, HD = 4, 2048, 1024, 16, 64
SCALE = HD ** -0.5
NCORES = 8
HPC = H // 2            # heads per core
PAIRS = HPC // 2        # head pairs per core
NKB = N // 128          # key blocks
NQC = N // 512          # query chunks
DC = D // 128           # contraction chunks over D
BF16 = ml_dtypes.bfloat16
E4M3 = ml_dtypes.float8_e4m3
WS = 64.0               # host pre-scale on Wq/Wk/Wv for the fp8 hi/lo split
ESCALE = SCALE / (WS * WS)  # exp() input scale undoing the WS^2 in q.k

_CACHE = {}


def _legalize_bir_waits(bir: bytes) -> bytes:
    """walrus in this container accepts at most ONE sync wait (and update)
    per instruction; Tile emits several. Split excess waits onto preceding
    same-engine NoOps (engines execute their stream in order, so a chain of
    single-wait NoOps is equivalent to one multi-wait instruction), and
    excess updates onto following same-engine NoOps."""
    import orjson

    m = orjson.loads(bir)
    ctr = 0
    for fn in m["functions"]:
        for bb in fn.get("blocks") or []:
            insts = bb.get("instructions")
            if not insts:
                continue
            out = []
            changed = False
            for inst in insts:
                si = inst.get("sync_info")
                eng = inst.get("engine")
                ow = (si or {}).get("on_wait") or []
                if len(ow) > 1 and eng and eng != "Unassigned":
                    for w in ow[:-1]:
                        ctr += 1
                        out.append(
                            {
                                "debug": inst.get("debug", 0),
                                "engine": eng,
                                "ins": [],
                                "name": f"{inst['name']}-lw{ctr}",
                                "opcode": "NoOp",
                                "outs": [],
                                "sync_info": {"on_update": [], "on_wait": [w]},
                            }
                        )
                    si["on_wait"] = [ow[-1]]
                    changed = True
                out.append(inst)
                ou = (si or {}).get("on_update") or []
                if len(ou) > 1 and eng and eng != "Unassigned":
                    for u in ou[1:]:
                        ctr += 1
                        out.append(
                            {
                                "debug": inst.get("debug", 0),
                                "engine": eng,
                                "ins": [],
                                "name": f"{inst['name']}-lu{ctr}",
                                "opcode": "NoOp",
                                "outs": [],
                                "sync_info": {"on_update": [u], "on_wait": []},
                            }
                        )
                    si["on_update"] = [ou[0]]
                    changed = True
            if changed:
                bb["instructions"] = out
    return orjson.dumps(m)


def _install_drain_patch():
    """Route every module serialization through the wait legalizer."""
    if _CACHE.get("drain_patched"):
        return
    import concourse.bass as bass

    orig = bass.Bass.to_json_bytes

    def patched(self):
        return _legalize_bir_waits(orig(self))

    bass.Bass.to_json_bytes = patched
    _CACHE["drain_patched"] = True


def _build_module():
    """Build the (single-NEFF, SPMD) Bass module for one core's work."""
    if "nc" in _CACHE:
        return _CACHE["nc"]
    _install_drain_patch()
    import concourse.bass as bass
    import concourse.mybir as mybir
    import concourse.tile as tile

    bf = mybir.dt.bfloat16
    f8 = mybir.dt.float8e4
    f32 = mybir.dt.float32
    EXP = mybir.ActivationFunctionType.Exp
    LN = mybir.ActivationFunctionType.Ln
    DR = mybir.MatmulPerfMode.DoubleRow

    nc = bass.Bass()
    # x split host-side into fp8 (lo, hi) grouped by 512-query column group;
    # wq/wk split into (hi, lo) grouped by head-pair block; wv by chunk.
    # Each group is contiguous per partition = one cheap DMA.
    xhl = nc.dram_tensor("xhl", (128, NQC, DC, 2, 512), f8, kind="ExternalInput")
    wqhl = nc.dram_tensor("wqhl", (128, PAIRS, DC, 2, 128), f8, kind="ExternalInput")
    wkhl = nc.dram_tensor("wkhl", (128, PAIRS, DC, 2, 128), f8, kind="ExternalInput")
    wvhl = nc.dram_tensor("wvhl", (128, DC, 2, 512), f8, kind="ExternalInput")
    wohl = nc.dram_tensor("wohl", (128, PAIRS, 2, D), f8, kind="ExternalInput")
    cmask = nc.dram_tensor("cmask", (128, 128), bf, kind="ExternalInput")
    outT = nc.dram_tensor("outT", (D, N), f32, kind="ExternalOutput")

    with tile.TileContext(nc) as tc:
        with (
            tc.tile_pool(name="const", bufs=1) as const,
            tc.tile_pool(name="work", bufs=3) as work,
            tc.tile_pool(name="ps", bufs=2, space="PSUM") as ps,
        ):
            # --- resident SBUF tensors ---------------------------------
            x_sb = const.tile([128, NQC, DC, 2, 512], f8, tag="x_sb", name="x_sb")
            wq_sb = const.tile([128, PAIRS, DC, 2, 128], f8, tag="wq_sb", name="wq_sb")
            wk_sb = const.tile([128, PAIRS, DC, 2, 128], f8, tag="wk_sb", name="wk_sb")
            wv_sb = const.tile([128, DC, 2, 512], f8, tag="wv_sb", name="wv_sb")
            wo_sb = const.tile([128, PAIRS, 2, D], f8, tag="wo_sb", name="wo_sb")
            qt_sb = const.tile([128, PAIRS, N], bf, tag="qt_sb", name="qt_sb")
            kt_sb = const.tile([128, PAIRS, N], bf, tag="kt_sb", name="kt_sb")
            # V augmented with 64 ones-columns: PV matmul output rows 64:127
            # all hold the P^T rowsum, physically replicated across partitions
            v_sb = const.tile([128, NKB, HPC, 128], bf, tag="v_sb", name="v_sb")
            o_sb = const.tile([128, PAIRS, N], bf, tag="o_sb", name="o_sb")
            # fp8 (lo, hi) split of o for the 3-term fp8 out-projection
            ohl_sb = const.tile([128, PAIRS, 2, N], f8, tag="ohl_sb", name="ohl_sb")
            mk_sb = const.tile([128, 128], bf, tag="mk_sb", name="mk_sb")

            # --- input DMAs, priority-ordered for the startup pipeline:
            # qk_proj(0, qc0) needs x group 0 + wq/wk mblk 0 (1.5MB), then
            # V(0..3) adds wv (1MB); later x groups / w blocks stream in
            # ahead of the attention chunks that consume them. Each DMA is
            # one contiguous per-partition run (cheap on HWDGE). ---------
            nc.sync.dma_start(out=wq_sb[:, 0], in_=wqhl[:, 0])
            for jq in range(0, DC, 2):
                nc.sync.dma_start(out=x_sb[:, 0, jq : jq + 2], in_=xhl[:, 0, jq : jq + 2])
            nc.sync.dma_start(out=wk_sb[:, 0], in_=wkhl[:, 0])
            nc.sync.dma_start(out=mk_sb, in_=cmask[:, :])
            nc.sync.dma_start(out=wv_sb[:, 0:4], in_=wvhl[:, 0:4, :, :])
            nc.sync.dma_start(out=wv_sb[:, 4:8], in_=wvhl[:, 4:8, :, :])
            for g in range(1, NQC):
                nc.sync.dma_start(out=x_sb[:, g], in_=xhl[:, g])
                nc.sync.dma_start(out=wq_sb[:, g], in_=wqhl[:, g])
                nc.sync.dma_start(out=wk_sb[:, g], in_=wkhl[:, g])
            for j in range(PAIRS):
                nc.sync.dma_start(out=wo_sb[:, j], in_=wohl[:, j])

            # Projections are emitted as lists of single-instruction steps
            # so they can either run as a burst (run_steps) or be spread
            # one instruction at a time through the attention kb loop,
            # filling the PE while the ACT engine works through the exp
            # stream (which is ~20% slower than QK+PV per key block).
            pend_hard = []   # projection steps needed by the next chunk
            pend_soft = []   # out-projection steps (loose deadline)

            def run_steps(steps):
                for s in steps:
                    s()

            def feed_hard(n):
                for _ in range(min(n, len(pend_hard))):
                    pend_hard.pop(0)()

            def feed_soft(n):
                for _ in range(min(n, len(pend_soft))):
                    pend_soft.pop(0)()

            # 3-term fp8 hi/lo projection: per chunk pair, one DoubleRow
            # matmul for hi*hi (2 k-tiles) plus one DoubleRow per chunk
            # pairing (lo, hi) x-tiles against (hi, lo) w-tiles for the two
            # cross terms. Step order follows chunk DMA arrival.
            def proj3_steps(lhs_of, rhs_of, drain, name):
                st = {}

                def pp():
                    if "t" not in st:
                        st["t"] = ps.tile([128, 512], f32, tag="proj", name=name, bufs=2)
                    return st["t"]

                def mm(j, main, start, stop):
                    def go():
                        nc.tensor.matmul(
                            pp(), lhsT=lhs_of(j, main), rhs=rhs_of(j, main),
                            start=start, stop=stop, perf_mode=DR,
                        )
                    return go

                steps = []
                first = True
                for j in range(0, DC, 2):
                    steps.append(mm(j, None, first, False))
                    first = False
                    steps.append(mm(j, 2, False, False))
                    steps.append(mm(j + 1, None, False, j + 1 == DC - 1))
                steps.append(lambda: drain(st["t"]))
                return steps

            # --- V projection for one 128-row seq block ----------------
            def v_proj_steps(sblk):
                cg, sc = divmod(sblk, 4)
                scol = slice(sc * 128, (sc + 1) * 128)

                def lhs_of(j, main):
                    if main:  # hi tiles of chunks j, j+1
                        return x_sb[:, cg, j : j + 2, 1, scol]
                    return x_sb[:, cg, j, :, scol]  # (lo, hi)

                def rhs_of(j, main):
                    if main:
                        return wv_sb[:, j : j + 2, 0, :]
                    return wv_sb[:, j, :, :]  # (hi, lo)

                def drain(vp):
                    nc.any.tensor_copy(
                        out=v_sb[:, sblk, :, 0:HD],
                        in_=vp.rearrange("p (h d) -> p h d", h=HPC),
                    )

                return proj3_steps(lhs_of, rhs_of, drain, "vp_ps")

            # --- Q^T / K^T projection for one (pair block, q chunk) ----
            def qk_proj_steps(mblk, qc):
                out = []
                for w_sb, dst in ((wq_sb, qt_sb), (wk_sb, kt_sb)):

                    def lhs_of(j, main, w_sb=w_sb):
                        if main:
                            return w_sb[:, mblk, j : j + 2, 0, :]
                        return w_sb[:, mblk, j, :, :]  # (hi, lo)

                    def rhs_of(j, main):
                        if main:
                            return x_sb[:, qc, j : j + 2, 1, :]
                        return x_sb[:, qc, j, :, :]  # (lo, hi)

                    def drain(pp, dst=dst):
                        nc.any.tensor_copy(
                            out=dst[:, mblk, qc * 512 : (qc + 1) * 512],
                            in_=pp,
                        )

                    out.extend(proj3_steps(lhs_of, rhs_of, drain, "qkproj_ps"))
                return out

            def next_stream_steps(a, qc):
                """Projections consumed by the stream after (a, qc)."""
                steps = []
                if a + 1 < PAIRS:
                    steps.extend(qk_proj_steps(a + 1, qc))
                elif qc + 1 < NQC:
                    for sblk in range(4 * (qc + 1), 4 * (qc + 1) + 4):
                        steps.extend(v_proj_steps(sblk))
                    steps.extend(qk_proj_steps(0, qc + 1))
                return steps

            # --- output projection for one q chunk (3-term fp8) --------
            # psum = (64 o_norm)*(64 Wo) = 4096*(o_norm . Wo); the 1/4096
            # descale folds into the drain copy.
            def out_proj_steps(qc):
                steps = []
                qs = slice(qc * 512, (qc + 1) * 512)
                for ob in range(8):
                    st = {}
                    obs = slice(ob * 128, (ob + 1) * 128)

                    def op(st=st):
                        if "t" not in st:
                            st["t"] = ps.tile([128, 512], f32, tag="proj", name="op_ps", bufs=2)
                        return st["t"]

                    def mm_main(a, start, obs=obs, op=op):
                        def go():
                            nc.tensor.matmul(
                                op(),
                                lhsT=wo_sb[:, a : a + 2, 0, obs],
                                rhs=ohl_sb[:, a : a + 2, 1, qs],
                                start=start, stop=False, perf_mode=DR,
                            )
                        return go

                    def mm_cross(a, stop, obs=obs, op=op):
                        def go():
                            nc.tensor.matmul(
                                op(),
                                lhsT=wo_sb[:, a, :, obs],
                                rhs=ohl_sb[:, a, :, qs],
                                start=False, stop=stop, perf_mode=DR,
                            )
                        return go

                    steps.append(mm_main(0, True))
                    steps.append(mm_cross(0, False))
                    steps.append(mm_cross(1, False))
                    steps.append(mm_main(2, False))
                    steps.append(mm_cross(2, False))
                    steps.append(mm_cross(3, True))

                    def drain(ob=ob, op=op):
                        oc = work.tile([128, 512], f32, tag="oc", name="oc", bufs=5)
                        nc.any.tensor_scalar_mul(oc, op(), 1.0 / 1024.0)
                        nc.sync.dma_start(
                            out=outT[ob * 128 : (ob + 1) * 128, qc * 512 : (qc + 1) * 512],
                            in_=oc,
                        )

                    steps.append(drain)
                return steps

            # --- attention --------------------------------------------
            # diagonal blocks kb = 4*qc + r only need queries q >= 128*r of
            # the 512-wide chunk (the rest is fully causal-masked): slice
            # QK/exp/PV to q in [128*r, 512) and apply a single 128x128
            # tril mask to the [128r, 128r+128) square.
            def emit_qk(a, qc, kb):
                r = kb - 4 * qc if kb >= 4 * qc else 0
                off = 128 * r
                qk = ps.tile([128, 1024], f32, tag="qk", name="qk_ps")
                for h in range(2):
                    nc.tensor.matmul(
                        qk[:, h * 512 + off : (h + 1) * 512],
                        lhsT=kt_sb[h * 64 : (h + 1) * 64, a, kb * 128 : (kb + 1) * 128],
                        rhs=qt_sb[h * 64 : (h + 1) * 64, a, qc * 512 + off : (qc + 1) * 512],
                        start=True,
                        stop=True,
                    )
                return qk

            norm_q = []

            def emit_norm(a, qc, ou, split=False):
                # ln(4r): the extra ln4 makes rec = 0.25/r, keeping the
                # fp8 o split (16*o_norm, extremes ~75) inside e4m3 range
                rec = work.tile([64, 1024], mybir.dt.float32, tag="rec", name="rec", bufs=2)
                qs = slice(qc * 512, (qc + 1) * 512)
                if not split:
                    nc.scalar.activation(out=rec, in_=ou[64:128, :], func=LN, scale=4.0)
                    nc.scalar.activation(out=rec, in_=rec, func=EXP, scale=-1.0)
                    for h in range(2):
                        nc.vector.tensor_tensor(
                            o_sb[h * 64 : (h + 1) * 64, a, qc * 512 : (qc + 1) * 512],
                            ou[0:HD, h * 512 : (h + 1) * 512],
                            rec[:, h * 512 : (h + 1) * 512],
                            mybir.AluOpType.mult,
                        )
                    nc.vector.tensor_copy(out=ohl_sb[:, a, 1, qs], in_=o_sb[:, a, qs])
                    nc.vector.tensor_sub(
                        ohl_sb[:, a, 0, qs], o_sb[:, a, qs], ohl_sb[:, a, 1, qs]
                    )
                    return
                # per-head pipeline for the final chunk: halves the tail's
                # serialized norm->extract chain by overlapping ACT and DVE
                for h in range(2):
                    hs = slice(h * 512, (h + 1) * 512)
                    hp = slice(h * 64, (h + 1) * 64)
                    hq = slice(qc * 512 + 0, (qc + 1) * 512)
                    nc.scalar.activation(out=rec[:, hs], in_=ou[64:128, hs], func=LN, scale=4.0)
                    nc.scalar.activation(out=rec[:, hs], in_=rec[:, hs], func=EXP, scale=-1.0)
                    nc.vector.tensor_tensor(
                        o_sb[hp, a, hq], ou[0:HD, hs], rec[:, hs], mybir.AluOpType.mult,
                    )
                    nc.vector.tensor_copy(out=ohl_sb[hp, a, 1, qs], in_=o_sb[hp, a, qs])
                    nc.vector.tensor_sub(
                        ohl_sb[hp, a, 0, qs], o_sb[hp, a, qs], ohl_sb[hp, a, 1, qs]
                    )

            # startup: pair-0 qc-0 prereqs as a burst (DMA-paced anyway)
            run_steps(qk_proj_steps(0, 0))
            # ones columns for the augmented-V rowsum trick (split across
            # Pool and DVE, emitted after the first proj so the early DVE
            # drains aren't queued behind the memset)
            nc.gpsimd.memset(v_sb[:, 0 : NKB // 2, :, HD:128], 1.0)
            nc.vector.memset(v_sb[:, NKB // 2 : NKB, :, HD:128], 1.0)
            for sblk in range(4):
                run_steps(v_proj_steps(sblk))
            # qc-outer / pair-inner: out-projection of chunk qc (and its
            # 2MB of output DMA) runs during chunk qc+1's attention, so the
            # output stream spreads across the whole run instead of piling
            # up behind the last pair.
            for qc in range(NQC):
                nkb = 4 * qc + 4
                for a in range(PAIRS):
                    # from qc>=1 the next stream's projections interleave
                    # into this chunk's kb loop (hard deadline: next chunk);
                    # during qc==0 the input DMAs are still streaming, so
                    # they burst at chunk end instead (legacy behavior)
                    if qc >= 1:
                        pend_hard.extend(next_stream_steps(a, qc))
                    # pv psum per head: half-sized tiles drain (and free)
                    # independently, halving the next chunk's PV wait
                    pvh = [
                        ps.tile([128, 512], f32, tag="pv", name="pv_ps", bufs=2)
                        for _ in range(2)
                    ]
                    qk_q = [emit_qk(a, qc, kb) for kb in range(min(2, nkb))]
                    for kb in range(nkb):
                        qk = qk_q.pop(0)
                        if kb + 2 < nkb:
                            qk_q.append(emit_qk(a, qc, kb + 2))
                        rem = max(1, nkb - 1 - kb)
                        feed_hard(-(-len(pend_hard) // rem))
                        if qc == NQC - 1 and a == PAIRS - 1:
                            feed_soft(2)
                        elif not pend_hard and len(pend_soft) > 8:
                            feed_soft(2)
                        r = kb - 4 * qc if kb >= 4 * qc else 0
                        off = 128 * r
                        pt = work.tile([128, 2, 512], bf, tag="pt", name="pt", bufs=4)
                        if r == 0:
                            nc.scalar.activation(
                                out=pt.rearrange("p h q -> p (h q)"),
                                in_=qk[:, :],
                                func=EXP,
                                scale=ESCALE,
                            )
                        else:
                            nc.scalar.activation(
                                out=pt[:, :, off:512],
                                in_=qk.rearrange("p (h q) -> p h q", h=2)[:, :, off:512],
                                func=EXP,
                                scale=ESCALE,
                            )
                        if kb >= 4 * qc:
                            nc.vector.tensor_mul(
                                pt[:, :, off : off + 128],
                                pt[:, :, off : off + 128],
                                mk_sb[:, None, :].to_broadcast([128, 2, 128]),
                            )
                        for h in range(2):
                            nc.tensor.matmul(
                                pvh[h][:, off:512],
                                lhsT=v_sb[:, kb, 2 * a + h, :],
                                rhs=pt[:, h, off:512],
                                start=(kb == 0),
                                stop=(kb == nkb - 1),
                                skip_group_check=True,
                            )
                    # copy PV psum to SBUF right away (frees the pv slots),
                    # but defer the normalization (ln/exp reciprocal +
                    # multiply) so the ACT reciprocal hides inside later
                    # exp streams
                    feed_hard(len(pend_hard))  # safety: next chunk needs these
                    ou = work.tile([128, 1024], mybir.dt.float32, tag="ou", name="ou", bufs=5)
                    for h in range(2):
                        nc.vector.tensor_copy(out=ou[:, h * 512 : (h + 1) * 512], in_=pvh[h])
                    norm_q.append((a, qc, ou))
                    if qc == 0:
                        # burst next stream's projections (see above)
                        run_steps(next_stream_steps(a, qc))
                    # drain the norm queue gradually through the last chunk
                    # so the final out-proj isn't gated on a norm cascade
                    # after the last PV
                    lag = (3 - a) if qc == NQC - 1 else 4
                    while len(norm_q) > lag:
                        na, nqc, nou = norm_q.pop(0)
                        emit_norm(na, nqc, nou)
                        if na == PAIRS - 1:
                            pend_soft.extend(out_proj_steps(nqc))
            while norm_q:
                na, nqc, nou = norm_q.pop(0)
                emit_norm(na, nqc, nou, split=not norm_q)
                if na == PAIRS - 1:
                    pend_soft.extend(out_proj_steps(nqc))
            feed_soft(len(pend_soft))

    _CACHE["nc"] = nc
    return nc


def _causal_masks():
    k = np.arange(128)[:, None]
    q = np.arange(128)[None, :]
    return (q >= k).astype(BF16)


def _split_pair(a, lo_first):
    """[D, M] f32 -> (lo/hi-stacked [2, DC, 128, M] fp8)."""
    ar = np.ascontiguousarray(a).reshape(DC, 128, -1)
    hi = ar.astype(E4M3)
    lo = (ar - hi.astype(np.float32)).astype(E4M3)
    pair = (lo, hi) if lo_first else (hi, lo)
    return np.stack(pair, axis=0)


def _split_x(a):
    """x.T [D, N] -> [128, NQC, DC, 2, 512] fp8 (lo, hi) by column group."""
    s = _split_pair(a, lo_first=True)  # [2, DC, 128, N]
    s = s.reshape(2, DC, 128, NQC, 512)
    return np.ascontiguousarray(s.transpose(2, 3, 1, 0, 4))


def _split_w(a):
    """W.T [D, 512] -> [128, PAIRS, DC, 2, 128] fp8 (hi, lo) by pair block."""
    s = _split_pair(a, lo_first=False)  # [2, DC, 128, 512]
    s = s.reshape(2, DC, 128, PAIRS, 128)
    return np.ascontiguousarray(s.transpose(2, 3, 1, 0, 4))


def _split_wv(a):
    """Wv.T [D, 512] -> [128, DC, 2, 512] fp8 (hi, lo) by chunk."""
    s = _split_pair(a, lo_first=False)  # [2, DC, 128, 512]
    return np.ascontiguousarray(s.transpose(2, 1, 0, 3))


def _split_wo(a):
    """Wo.T slice [512, D] -> [128, PAIRS, 2, D] fp8 (hi, lo) by pair."""
    ar = np.ascontiguousarray(a).reshape(PAIRS, 128, D)
    hi = ar.astype(E4M3)
    lo = (ar - hi.astype(np.float32)).astype(E4M3)
    s = np.stack((hi, lo), axis=0)  # [2, PAIRS, 128, D]
    return np.ascontiguousarray(s.transpose(2, 1, 0, 3))


def _prep_in_maps(x, Wq, Wk, Wv, Wo):
    cm = _causal_masks()
    in_maps = []
    xhl_b = [_split_x(x[b].T) for b in range(B)]
    for c in range(NCORES):
        b, hg = divmod(c, 2)
        rs = slice(hg * 512, (hg + 1) * 512)
        in_maps.append(
            {
                "xhl": xhl_b[b],
                "wqhl": _split_w(Wq[rs].T * WS),
                "wkhl": _split_w(Wk[rs].T * WS),
                "wvhl": _split_wv(Wv[rs].T * WS),
                "wohl": _split_wo(Wo[:, rs].T * WS),
                "cmask": cm,
            }
        )
    return in_maps


def _is_causal(mask):
    mask = np.asarray(mask)
    if mask.shape != (N, N):
        return False
    return bool(np.array_equal(mask, np.tril(np.ones((N, N), dtype=bool))))


def _numpy_fallback(x, mask, Wq, Wk, Wv, Wo):
    out = np.empty((B, N, D), np.float32)
    madd = np.where(np.asarray(mask), 0.0, -np.inf).astype(np.float32)
    for b in range(B):
        q = (x[b] @ Wq.T).reshape(N, H, HD).transpose(1, 0, 2)
        k = (x[b] @ Wk.T).reshape(N, H, HD).transpose(1, 0, 2)
        v = (x[b] @ Wv.T).reshape(N, H, HD).transpose(1, 0, 2)
        o = np.empty((H, N, HD), np.float32)
        for h in range(H):
            s = q[h] @ k[h].T * SCALE + madd
            s -= s.max(axis=-1, keepdims=True)
            p = np.exp(s)
            p /= p.sum(axis=-1, keepdims=True)
            o[h] = p @ v[h]
        out[b] = o.transpose(1, 0, 2).reshape(N, D) @ Wo.T
    return out


def _run_device(x, Wq, Wk, Wv, Wo):
    from concourse.bass_utils import run_bass_kernel_spmd

    nc = _build_module()
    in_maps = _prep_in_maps(x, Wq, Wk, Wv, Wo)
    res = run_bass_kernel_spmd(nc, in_maps, core_ids=list(range(NCORES)))
    outs = [r["outT"] for r in res.results]
    out = np.empty((B, N, D), np.float32)
    for b in range(B):
        out[b] = (outs[2 * b] + outs[2 * b + 1]).T
    return out


def kernel(x, mask, Wq, Wk, Wv, Wo):
    x = np.asarray(x, dtype=np.float32)
    Wq = np.asarray(Wq, dtype=np.float32)
    Wk = np.asarray(Wk, dtype=np.float32)
    Wv = np.asarray(Wv, dtype=np.float32)
    Wo = np.asarray(Wo, dtype=np.float32)
    if not _is_causal(mask):
        return _numpy_fallback(x, mask, Wq, Wk, Wv, Wo)
    try:
        return _run_device(x, Wq, Wk, Wv, Wo)
    except Exception:
        try:
            return _run_device(x, Wq, Wk, Wv, Wo)
        except Exception:
            # last resort: slow but correct
            return _numpy_fallback(x, mask, Wq, Wk, Wv, Wo)


def simulate():
    """Cost-model timeline estimate of one core's NEFF execution (ns)."""
    from concourse.timeline_sim import TimelineSim

    nc = _build_module()
    return TimelineSim(nc).simulate()



# revision 60
# speedup vs baseline: 1.0005x; 1.0005x over previous
"""Multi-head causal attention (B=4, N=2048, D=1024, H=16, d=64) on 8 TRN2 cores.

Sharding: core c handles batch b = c//2 and head-group hg = c%2 (8 heads).
Each core computes Q/K/V projections for its heads, causal flash-style
attention, and a partial output projection; the host sums the two partials
per batch (all-reduce done host-side) and transposes back.

All four projections run as 3-term fp8e4m3 hi/lo DoubleRow matmuls at 0.75x
the bf16 PE cost with better-than-bf16 accuracy: operands are split into
fp8 hi + unscaled fp8 lo residual (x/W splits on the host for free; the
attention output's split on the DVE), and each psum accumulates
  hi.hi (two 128-chunks per DoubleRow instr) + (lo.hi + hi.lo) (the two
  cross terms as the two k-tiles of one DoubleRow instr), dropping lo.lo.
W's are pre-scaled by WS=64 (descale folded into the exp scale / the final
copy); the attention-output split is kept at 16*o_norm via an extra ln(4)
in the reciprocal so extremes stay inside e4m3 range. Raw fp8 (un-split)
fails the 2e-2 budget everywhere (proj 6.6e-2, QK 4.5e-2, PV 3.1e-2 even
renormalized), so QK/PV stay bf16 -- both are at their structural PE floor
(QK output-bound at M=128, PV contraction-bound at 128 keys).

Attention per (q-chunk, head-pair), transposed layouts throughout:
  S^T = K Q^T per 128-key block (lhsT=KT, K=64, 2 heads side by side in F)
  P^T = exp(S^T * SCALE/WS^2) on ACT straight from PSUM
  rowsums via ones-augmented V (V'' = [V | 1*64], M=128) during the PV
        matmul -- PSUM rows 64:127 hold the rowsum replicated across
        partitions, so no partition-broadcast is ever needed
  0.25/rowsum = exp(-ln(4*rowsum)) on ACT, deferred 4 chunks so it hides
        inside later exp streams.

Schedule: the exp stream (1 elem/partition/cycle on ACT) is ~22% slower
than the QK+PV PE work per key block, so projections are queued as
single-instruction steps and fed into the kb loop between the QK prefetch
and the PV, keeping the PE busy while exp cooks ("pend_hard" = next
stream's projections, deadline-spread; "pend_soft" = out-projections,
fed when slack). Loops run qc-outer so each chunk's out-projection (and
its 2MB output DMA) overlaps the next chunk's attention. Input DMAs are
contiguous-per-partition groups (128 descriptors each), priority-ordered
so the 1.5MB needed by qk_proj(0,0) lands first.

Pitfalls encoded here: GPSIMD cannot touch PSUM (all psum drains go via
nc.any to ACT/DVE); DVE has no float divide; fp8 stores do not saturate
(overflow => inf => NaN); multi-wait instructions are re-legalized into
single-wait NoOp chains for this container's walrus.
"""

import sys

import numpy as np

if "/opt/trn_rl_repo" not in sys.path:
    sys.path.insert(0, "/opt/trn_rl_repo")

import ml_dtypes

B, N, D, H, HD = 4, 2048, 1024, 16, 64
SCALE = HD ** -0.5
NCORES = 8
HPC = H // 2            # heads per core
PAIRS = HPC // 2        # head pairs per core
NKB = N // 128          # key blocks
NQC = N // 512          # query chunks
DC = D // 128           # contraction chunks over D
BF16 = ml_dtypes.bfloat16
E4M3 = ml_dtypes.float8_e4m3
WS = 64.0               # host pre-scale on Wq/Wk/Wv for the fp8 hi/lo split
ESCALE = SCALE / (WS * WS)  # exp() input scale undoing the WS^2 in q.k

_CACHE = {}


def _legalize_bir_waits(bir: bytes) -> bytes:
    """walrus in this container accepts at most ONE sync wait (and update)
    per instruction; Tile emits several. Split excess waits onto preceding
    same-engine NoOps (engines execute their stream in order, so a chain of
    single-wait NoOps is equivalent to one multi-wait instruction), and
    excess updates onto following same-engine NoOps."""
    import orjson

    m = orjson.loads(bir)
    ctr = 0
    for fn in m["functions"]:
        for bb in fn.get("blocks") or []:
            insts = bb.get("instructions")
            if not insts:
                continue
            out = []
            changed = False
            for inst in insts:
                si = inst.get("sync_info")
                eng = inst.get("engine")
                ow = (si or {}).get("on_wait") or []
                if len(ow) > 1 and eng and eng != "Unassigned":
                    for w in ow[:-1]:
                        ctr += 1
                        out.append(
                            {
                                "debug": inst.get("debug", 0),
                                "engine": eng,
                                "ins": [],
                                "name": f"{inst['name']}-lw{ctr}",
                                "opcode": "NoOp",
                                "outs": [],
                                "sync_info": {"on_update": [], "on_wait": [w]},
                            }
                        )
                    si["on_wait"] = [ow[-1]]
                    changed = True
                out.append(inst)
                ou = (si or {}).get("on_update") or []
                if len(ou) > 1 and eng and eng != "Unassigned":
                    for u in ou[1:]:
                        ctr += 1
                        out.append(
                            {
                                "debug": inst.get("debug", 0),
                                "engine": eng,
                                "ins": [],
                                "name": f"{inst['name']}-lu{ctr}",
                                "opcode": "NoOp",
                                "outs": [],
                                "sync_info": {"on_update": [u], "on_wait": []},
                            }
                        )
                    si["on_update"] = [ou[0]]
                    changed = True
            if changed:
                bb["instructions"] = out
    return orjson.dumps(m)


def _install_drain_patch():
    """Route every module serialization through the wait legalizer."""
    if _CACHE.get("drain_patched"):
        return
    import concourse.bass as bass

    orig = bass.Bass.to_json_bytes

    def patched(self):
        return _legalize_bir_waits(orig(self))

    bass.Bass.to_json_bytes = patched
    _CACHE["drain_patched"] = True


def _build_module():
    """Build the (single-NEFF, SPMD) Bass module for one core's work."""
    if "nc" in _CACHE:
        return _CACHE["nc"]
    _install_drain_patch()
    import concourse.bass as bass
    import concourse.mybir as mybir
    import concourse.tile as tile

    bf = mybir.dt.bfloat16
    f8 = mybir.dt.float8e4
    f32 = mybir.dt.float32
    EXP = mybir.ActivationFunctionType.Exp
    LN = mybir.ActivationFunctionType.Ln
    DR = mybir.MatmulPerfMode.DoubleRow

    nc = bass.Bass()
    # x split host-side into fp8 (lo, hi) grouped by 512-query column group;
    # wq/wk split into (hi, lo) grouped by head-pair block; wv by chunk.
    # Each group is contiguous per partition = one cheap DMA.
    xhl = nc.dram_tensor("xhl", (128, NQC, DC, 2, 512), f8, kind="ExternalInput")
    wqhl = nc.dram_tensor("wqhl", (128, PAIRS, DC, 2, 128), f8, kind="ExternalInput")
    wkhl = nc.dram_tensor("wkhl", (128, PAIRS, DC, 2, 128), f8, kind="ExternalInput")
    wvhl = nc.dram_tensor("wvhl", (128, DC, 2, 512), f8, kind="ExternalInput")
    wohl = nc.dram_tensor("wohl", (128, PAIRS, 2, D), f8, kind="ExternalInput")
    cmask = nc.dram_tensor("cmask", (128, 128), bf, kind="ExternalInput")
    outT = nc.dram_tensor("outT", (D, N), f32, kind="ExternalOutput")

    with tile.TileContext(nc) as tc:
        with (
            tc.tile_pool(name="const", bufs=1) as const,
            tc.tile_pool(name="work", bufs=3) as work,
            tc.tile_pool(name="ps", bufs=2, space="PSUM") as ps,
        ):
            # --- resident SBUF tensors ---------------------------------
            x_sb = const.tile([128, NQC, DC, 2, 512], f8, tag="x_sb", name="x_sb")
            wq_sb = const.tile([128, PAIRS, DC, 2, 128], f8, tag="wq_sb", name="wq_sb")
            wk_sb = const.tile([128, PAIRS, DC, 2, 128], f8, tag="wk_sb", name="wk_sb")
            wv_sb = const.tile([128, DC, 2, 512], f8, tag="wv_sb", name="wv_sb")
            wo_sb = const.tile([128, PAIRS, 2, D], f8, tag="wo_sb", name="wo_sb")
            qt_sb = const.tile([128, PAIRS, N], bf, tag="qt_sb", name="qt_sb")
            kt_sb = const.tile([128, PAIRS, N], bf, tag="kt_sb", name="kt_sb")
            # V augmented with 64 ones-columns: PV matmul output rows 64:127
            # all hold the P^T rowsum, physically replicated across partitions
            v_sb = const.tile([128, NKB, HPC, 128], bf, tag="v_sb", name="v_sb")
            o_sb = const.tile([128, PAIRS, N], bf, tag="o_sb", name="o_sb")
            # fp8 (lo, hi) split of o for the 3-term fp8 out-projection
            ohl_sb = const.tile([128, PAIRS, 2, N], f8, tag="ohl_sb", name="ohl_sb")
            mk_sb = const.tile([128, 128], bf, tag="mk_sb", name="mk_sb")

            # --- input DMAs, priority-ordered for the startup pipeline:
            # qk_proj(0, qc0) needs x group 0 + wq/wk mblk 0 (1.5MB), then
            # V(0..3) adds wv (1MB); later x groups / w blocks stream in
            # ahead of the attention chunks that consume them. Each DMA is
            # one contiguous per-partition run (cheap on HWDGE). ---------
            nc.sync.dma_start(out=wq_sb[:, 0], in_=wqhl[:, 0])
            for jq in range(0, DC, 2):
                nc.sync.dma_start(out=x_sb[:, 0, jq : jq + 2], in_=xhl[:, 0, jq : jq + 2])
            nc.sync.dma_start(out=wk_sb[:, 0], in_=wkhl[:, 0])
            nc.sync.dma_start(out=mk_sb, in_=cmask[:, :])
            nc.sync.dma_start(out=wv_sb[:, 0:4], in_=wvhl[:, 0:4, :, :])
            nc.sync.dma_start(out=wv_sb[:, 4:8], in_=wvhl[:, 4:8, :, :])
            for g in range(1, NQC):
                nc.sync.dma_start(out=x_sb[:, g], in_=xhl[:, g])
                nc.sync.dma_start(out=wq_sb[:, g], in_=wqhl[:, g])
                nc.sync.dma_start(out=wk_sb[:, g], in_=wkhl[:, g])
            for j in range(PAIRS):
                nc.sync.dma_start(out=wo_sb[:, j], in_=wohl[:, j])

            # Projections are emitted as lists of single-instruction steps
            # so they can either run as a burst (run_steps) or be spread
            # one instruction at a time through the attention kb loop,
            # filling the PE while the ACT engine works through the exp
            # stream (which is ~20% slower than QK+PV per key block).
            pend_hard = []   # projection steps needed by the next chunk
            pend_soft = []   # out-projection steps (loose deadline)

            def run_steps(steps):
                for s in steps:
                    s()

            def feed_hard(n):
                for _ in range(min(n, len(pend_hard))):
                    pend_hard.pop(0)()

            def feed_soft(n):
                for _ in range(min(n, len(pend_soft))):
                    pend_soft.pop(0)()

            # 3-term fp8 hi/lo projection: per chunk pair, one DoubleRow
            # matmul for hi*hi (2 k-tiles) plus one DoubleRow per chunk
            # pairing (lo, hi) x-tiles against (hi, lo) w-tiles for the two
            # cross terms. Step order follows chunk DMA arrival.
            def proj3_steps(lhs_of, rhs_of, drain, name):
                st = {}

                def pp():
                    if "t" not in st:
                        st["t"] = ps.tile([128, 512], f32, tag="proj", name=name, bufs=2)
                    return st["t"]

                def mm(j, main, start, stop):
                    def go():
                        nc.tensor.matmul(
                            pp(), lhsT=lhs_of(j, main), rhs=rhs_of(j, main),
                            start=start, stop=stop, perf_mode=DR,
                        )
                    return go

                steps = []
                first = True
                for j in range(0, DC, 2):
                    steps.append(mm(j, None, first, False))
                    first = False
                    steps.append(mm(j, 2, False, False))
                    steps.append(mm(j + 1, None, False, j + 1 == DC - 1))
                steps.append(lambda: drain(st["t"]))
                return steps

            # --- V projection for one 128-row seq block ----------------
            def v_proj_steps(sblk):
                cg, sc = divmod(sblk, 4)
                scol = slice(sc * 128, (sc + 1) * 128)

                def lhs_of(j, main):
                    if main:  # hi tiles of chunks j, j+1
                        return x_sb[:, cg, j : j + 2, 1, scol]
                    return x_sb[:, cg, j, :, scol]  # (lo, hi)

                def rhs_of(j, main):
                    if main:
                        return wv_sb[:, j : j + 2, 0, :]
                    return wv_sb[:, j, :, :]  # (hi, lo)

                def drain(vp):
                    nc.any.tensor_copy(
                        out=v_sb[:, sblk, :, 0:HD],
                        in_=vp.rearrange("p (h d) -> p h d", h=HPC),
                    )

                return proj3_steps(lhs_of, rhs_of, drain, "vp_ps")

            # --- Q^T / K^T projection for one (pair block, q chunk) ----
            def qk_proj_steps(mblk, qc):
                out = []
                for w_sb, dst in ((wq_sb, qt_sb), (wk_sb, kt_sb)):

                    def lhs_of(j, main, w_sb=w_sb):
                        if main:
                            return w_sb[:, mblk, j : j + 2, 0, :]
                        return w_sb[:, mblk, j, :, :]  # (hi, lo)

                    def rhs_of(j, main):
                        if main:
                            return x_sb[:, qc, j : j + 2, 1, :]
                        return x_sb[:, qc, j, :, :]  # (lo, hi)

                    def drain(pp, dst=dst):
                        nc.any.tensor_copy(
                            out=dst[:, mblk, qc * 512 : (qc + 1) * 512],
                            in_=pp,
                        )

                    out.extend(proj3_steps(lhs_of, rhs_of, drain, "qkproj_ps"))
                return out

            def next_stream_steps(a, qc):
                """Projections consumed by the stream after (a, qc)."""
                steps = []
                if a + 1 < PAIRS:
                    steps.extend(qk_proj_steps(a + 1, qc))
                elif qc + 1 < NQC:
                    for sblk in range(4 * (qc + 1), 4 * (qc + 1) + 4):
                        steps.extend(v_proj_steps(sblk))
                    steps.extend(qk_proj_steps(0, qc + 1))
                return steps

            # --- output projection for one q chunk (3-term fp8) --------
            # psum = (64 o_norm)*(64 Wo) = 4096*(o_norm . Wo); the 1/4096
            # descale folds into the drain copy.
            def out_proj_steps(qc):
                steps = []
                qs = slice(qc * 512, (qc + 1) * 512)
                for ob in range(8):
                    st = {}
                    obs = slice(ob * 128, (ob + 1) * 128)

                    def op(st=st):
                        if "t" not in st:
                            st["t"] = ps.tile([128, 512], f32, tag="proj", name="op_ps", bufs=2)
                        return st["t"]

                    def mm_main(a, start, obs=obs, op=op):
                        def go():
                            nc.tensor.matmul(
                                op(),
                                lhsT=wo_sb[:, a : a + 2, 0, obs],
                                rhs=ohl_sb[:, a : a + 2, 1, qs],
                                start=start, stop=False, perf_mode=DR,
                            )
                        return go

                    def mm_cross(a, stop, obs=obs, op=op):
                        def go():
                            nc.tensor.matmul(
                                op(),
                                lhsT=wo_sb[:, a, :, obs],
                                rhs=ohl_sb[:, a, :, qs],
                                start=False, stop=stop, perf_mode=DR,
                            )
                        return go

                    # a3-dependent matmuls (main2 pairs a2+a3, cross3) go
                    # last so the in-order PE can pre-run the rest while the
                    # final pair's norm/extraction completes
                    steps.append(mm_main(0, True))
                    steps.append(mm_cross(0, False))
                    steps.append(mm_cross(1, False))
                    steps.append(mm_cross(2, False))
                    steps.append(mm_main(2, False))
                    steps.append(mm_cross(3, True))

                    def drain(ob=ob, op=op):
                        oc = work.tile([128, 512], f32, tag="oc", name="oc", bufs=5)
                        nc.any.tensor_scalar_mul(oc, op(), 1.0 / 1024.0)
                        nc.sync.dma_start(
                            out=outT[ob * 128 : (ob + 1) * 128, qc * 512 : (qc + 1) * 512],
                            in_=oc,
                        )

                    steps.append(drain)
                return steps

            # --- attention --------------------------------------------
            # diagonal blocks kb = 4*qc + r only need queries q >= 128*r of
            # the 512-wide chunk (the rest is fully causal-masked): slice
            # QK/exp/PV to q in [128*r, 512) and apply a single 128x128
            # tril mask to the [128r, 128r+128) square.
            def emit_qk(a, qc, kb):
                r = kb - 4 * qc if kb >= 4 * qc else 0
                off = 128 * r
                qk = ps.tile([128, 1024], f32, tag="qk", name="qk_ps")
                for h in range(2):
                    nc.tensor.matmul(
                        qk[:, h * 512 + off : (h + 1) * 512],
                        lhsT=kt_sb[h * 64 : (h + 1) * 64, a, kb * 128 : (kb + 1) * 128],
                        rhs=qt_sb[h * 64 : (h + 1) * 64, a, qc * 512 + off : (qc + 1) * 512],
                        start=True,
                        stop=True,
                    )
                return qk

            norm_q = []

            def emit_norm(a, qc, ou, split=False):
                # ln(4r): the extra ln4 makes rec = 0.25/r, keeping the
                # fp8 o split (16*o_norm, extremes ~75) inside e4m3 range
                rec = work.tile([64, 1024], mybir.dt.float32, tag="rec", name="rec", bufs=2)
                qs = slice(qc * 512, (qc + 1) * 512)
                if not split:
                    nc.scalar.activation(out=rec, in_=ou[64:128, :], func=LN, scale=4.0)
                    nc.scalar.activation(out=rec, in_=rec, func=EXP, scale=-1.0)
                    for h in range(2):
                        nc.vector.tensor_tensor(
                            o_sb[h * 64 : (h + 1) * 64, a, qc * 512 : (qc + 1) * 512],
                            ou[0:HD, h * 512 : (h + 1) * 512],
                            rec[:, h * 512 : (h + 1) * 512],
                            mybir.AluOpType.mult,
                        )
                    nc.vector.tensor_copy(out=ohl_sb[:, a, 1, qs], in_=o_sb[:, a, qs])
                    nc.vector.tensor_sub(
                        ohl_sb[:, a, 0, qs], o_sb[:, a, qs], ohl_sb[:, a, 1, qs]
                    )
                    return
                # per-head pipeline for the final chunk: halves the tail's
                # serialized norm->extract chain by overlapping ACT and DVE
                for h in range(2):
                    hs = slice(h * 512, (h + 1) * 512)
                    hp = slice(h * 64, (h + 1) * 64)
                    hq = slice(qc * 512 + 0, (qc + 1) * 512)
                    nc.scalar.activation(out=rec[:, hs], in_=ou[64:128, hs], func=LN, scale=4.0)
                    nc.scalar.activation(out=rec[:, hs], in_=rec[:, hs], func=EXP, scale=-1.0)
                    nc.vector.tensor_tensor(
                        o_sb[hp, a, hq], ou[0:HD, hs], rec[:, hs], mybir.AluOpType.mult,
                    )
                    nc.vector.tensor_copy(out=ohl_sb[hp, a, 1, qs], in_=o_sb[hp, a, qs])
                    nc.vector.tensor_sub(
                        ohl_sb[hp, a, 0, qs], o_sb[hp, a, qs], ohl_sb[hp, a, 1, qs]
                    )

            # startup: pair-0 qc-0 prereqs as a burst (DMA-paced anyway)
            run_steps(qk_proj_steps(0, 0))
            # ones columns for the augmented-V rowsum trick (split across
            # Pool and DVE, emitted after the first proj so the early DVE
            # drains aren't queued behind the memset)
            nc.gpsimd.memset(v_sb[:, 0 : NKB // 2, :, HD:128], 1.0)
            nc.vector.memset(v_sb[:, NKB // 2 : NKB, :, HD:128], 1.0)
            for sblk in range(4):
                run_steps(v_proj_steps(sblk))
            # qc-outer / pair-inner: out-projection of chunk qc (and its
            # 2MB of output DMA) runs during chunk qc+1's attention, so the
            # output stream spreads across the whole run instead of piling
            # up behind the last pair.
            for qc in range(NQC):
                nkb = 4 * qc + 4
                for a in range(PAIRS):
                    # from qc>=1 the next stream's projections interleave
                    # into this chunk's kb loop (hard deadline: next chunk);
                    # during qc==0 the input DMAs are still streaming, so
                    # they burst at chunk end instead (legacy behavior)
                    if qc >= 1:
                        pend_hard.extend(next_stream_steps(a, qc))
                    # pv psum per head: half-sized tiles drain (and free)
                    # independently, halving the next chunk's PV wait
                    pvh = [
                        ps.tile([128, 512], f32, tag="pv", name="pv_ps", bufs=2)
                        for _ in range(2)
                    ]
                    qk_q = [emit_qk(a, qc, kb) for kb in range(min(2, nkb))]
                    for kb in range(nkb):
                        qk = qk_q.pop(0)
                        if kb + 2 < nkb:
                            qk_q.append(emit_qk(a, qc, kb + 2))
                        rem = max(1, nkb - 1 - kb)
                        feed_hard(-(-len(pend_hard) // rem))
                        if qc == NQC - 1 and a == PAIRS - 1:
                            feed_soft(2)
                        elif not pend_hard and len(pend_soft) > 8:
                            feed_soft(2)
                        r = kb - 4 * qc if kb >= 4 * qc else 0
                        off = 128 * r
                        pt = work.tile([128, 2, 512], bf, tag="pt", name="pt", bufs=4)
                        if r == 0:
                            nc.scalar.activation(
                                out=pt.rearrange("p h q -> p (h q)"),
                                in_=qk[:, :],
                                func=EXP,
                                scale=ESCALE,
                            )
                        else:
                            nc.scalar.activation(
                                out=pt[:, :, off:512],
                                in_=qk.rearrange("p (h q) -> p h q", h=2)[:, :, off:512],
                                func=EXP,
                                scale=ESCALE,
                            )
                        if kb >= 4 * qc:
                            nc.vector.tensor_mul(
                                pt[:, :, off : off + 128],
                                pt[:, :, off : off + 128],
                                mk_sb[:, None, :].to_broadcast([128, 2, 128]),
                            )
                        for h in range(2):
                            nc.tensor.matmul(
                                pvh[h][:, off:512],
                                lhsT=v_sb[:, kb, 2 * a + h, :],
                                rhs=pt[:, h, off:512],
                                start=(kb == 0),
                                stop=(kb == nkb - 1),
                                skip_group_check=True,
                            )
                    # copy PV psum to SBUF right away (frees the pv slots),
                    # but defer the normalization (ln/exp reciprocal +
                    # multiply) so the ACT reciprocal hides inside later
                    # exp streams
                    feed_hard(len(pend_hard))  # safety: next chunk needs these
                    ou = work.tile([128, 1024], mybir.dt.float32, tag="ou", name="ou", bufs=5)
                    for h in range(2):
                        nc.vector.tensor_copy(out=ou[:, h * 512 : (h + 1) * 512], in_=pvh[h])
                    norm_q.append((a, qc, ou))
                    if qc == 0:
                        # burst next stream's projections (see above)
                        run_steps(next_stream_steps(a, qc))
                    # drain the norm queue gradually through the last chunk
                    # so the final out-proj isn't gated on a norm cascade
                    # after the last PV
                    lag = (3 - a) if qc == NQC - 1 else 4
                    while len(norm_q) > lag:
                        na, nqc, nou = norm_q.pop(0)
                        emit_norm(na, nqc, nou)
                        if na == PAIRS - 1:
                            pend_soft.extend(out_proj_steps(nqc))
            while norm_q:
                na, nqc, nou = norm_q.pop(0)
                emit_norm(na, nqc, nou, split=not norm_q)
                if na == PAIRS - 1:
                    pend_soft.extend(out_proj_steps(nqc))
            feed_soft(len(pend_soft))

    _CACHE["nc"] = nc
    return nc


def _causal_masks():
    k = np.arange(128)[:, None]
    q = np.arange(128)[None, :]
    return (q >= k).astype(BF16)


def _split_pair(a, lo_first):
    """[D, M] f32 -> (lo/hi-stacked [2, DC, 128, M] fp8)."""
    ar = np.ascontiguousarray(a).reshape(DC, 128, -1)
    hi = ar.astype(E4M3)
    lo = (ar - hi.astype(np.float32)).astype(E4M3)
    pair = (lo, hi) if lo_first else (hi, lo)
    return np.stack(pair, axis=0)


def _split_x(a):
    """x.T [D, N] -> [128, NQC, DC, 2, 512] fp8 (lo, hi) by column group."""
    s = _split_pair(a, lo_first=True)  # [2, DC, 128, N]
    s = s.reshape(2, DC, 128, NQC, 512)
    return np.ascontiguousarray(s.transpose(2, 3, 1, 0, 4))


def _split_w(a):
    """W.T [D, 512] -> [128, PAIRS, DC, 2, 128] fp8 (hi, lo) by pair block."""
    s = _split_pair(a, lo_first=False)  # [2, DC, 128, 512]
    s = s.reshape(2, DC, 128, PAIRS, 128)
    return np.ascontiguousarray(s.transpose(2, 3, 1, 0, 4))


def _split_wv(a):
    """Wv.T [D, 512] -> [128, DC, 2, 512] fp8 (hi, lo) by chunk."""
    s = _split_pair(a, lo_first=False)  # [2, DC, 128, 512]
    return np.ascontiguousarray(s.transpose(2, 1, 0, 3))


def _split_wo(a):
    """Wo.T slice [512, D] -> [128, PAIRS, 2, D] fp8 (hi, lo) by pair."""
    ar = np.ascontiguousarray(a).reshape(PAIRS, 128, D)
    hi = ar.astype(E4M3)
    lo = (ar - hi.astype(np.float32)).astype(E4M3)
    s = np.stack((hi, lo), axis=0)  # [2, PAIRS, 128, D]
    return np.ascontiguousarray(s.transpose(2, 1, 0, 3))


def _prep_in_maps(x, Wq, Wk, Wv, Wo):
    cm = _causal_masks()
    in_maps = []
    xhl_b = [_split_x(x[b].T) for b in range(B)]
    for c in range(NCORES):
        b, hg = divmod(c, 2)
        rs = slice(hg * 512, (hg + 1) * 512)
        in_maps.append(
            {
                "xhl": xhl_b[b],
                "wqhl": _split_w(Wq[rs].T * WS),
                "wkhl": _split_w(Wk[rs].T * WS),
                "wvhl": _split_wv(Wv[rs].T * WS),
                "wohl": _split_wo(Wo[:, rs].T * WS),
                "cmask": cm,
            }
        )
    return in_maps


def _is_causal(mask):
    mask = np.asarray(mask)
    if mask.shape != (N, N):
        return False
    return bool(np.array_equal(mask, np.tril(np.ones((N, N), dtype=bool))))


def _numpy_fallback(x, mask, Wq, Wk, Wv, Wo):
    out = np.empty((B, N, D), np.float32)
    madd = np.where(np.asarray(mask), 0.0, -np.inf).astype(np.float32)
    for b in range(B):
        q = (x[b] @ Wq.T).reshape(N, H, HD).transpose(1, 0, 2)
        k = (x[b] @ Wk.T).reshape(N, H, HD).transpose(1, 0, 2)
        v = (x[b] @ Wv.T).reshape(N, H, HD).transpose(1, 0, 2)
        o = np.empty((H, N, HD), np.float32)
        for h in range(H):
            s = q[h] @ k[h].T * SCALE + madd
            s -= s.max(axis=-1, keepdims=True)
            p = np.exp(s)
            p /= p.sum(axis=-1, keepdims=True)
            o[h] = p @ v[h]
        out[b] = o.transpose(1, 0, 2).reshape(N, D) @ Wo.T
    return out


def _run_device(x, Wq, Wk, Wv, Wo):
    from concourse.bass_utils import run_bass_kernel_spmd

    nc = _build_module()
    in_maps = _prep_in_maps(x, Wq, Wk, Wv, Wo)
    res = run_bass_kernel_spmd(nc, in_maps, core_ids=list(range(NCORES)))
    outs = [r["outT"] for r in res.results]
    out = np.empty((B, N, D), np.float32)
    for b in range(B):
        out[b] = (outs[2 * b] + outs[2 * b + 1]).T
    return out


def kernel(x, mask, Wq, Wk, Wv, Wo):
    x = np.asarray(x, dtype=np.float32)
    Wq = np.asarray(Wq, dtype=np.float32)
    Wk = np.asarray(Wk, dtype=np.float32)
    Wv = np.asarray(Wv, dtype=np.float32)
    Wo = np.asarray(Wo, dtype=np.float32)
    if not _is_causal(mask):
        return _numpy_fallback(x, mask, Wq, Wk, Wv, Wo)
    try:
        return _run_device(x, Wq, Wk, Wv, Wo)
    except Exception:
        try:
            return _run_device(x, Wq, Wk, Wv, Wo)
        except Exception:
            # last resort: slow but correct
            return _numpy_fallback(x, mask, Wq, Wk, Wv, Wo)


def simulate():
    """Cost-model timeline estimate of one core's NEFF execution (ns)."""
    from concourse.timeline_sim import TimelineSim

    nc = _build_module()
    return TimelineSim(nc).simulate()



# revision 67
# speedup vs baseline: 1.0025x; 1.0020x over previous
"""Multi-head causal attention (B=4, N=2048, D=1024, H=16, d=64) on 8 TRN2 cores.

Sharding: core c handles batch b = c//2 and head-group hg = c%2 (8 heads).
Each core computes Q/K/V projections for its heads, causal flash-style
attention, and a partial output projection; the host sums the two partials
per batch (all-reduce done host-side) and transposes back.

All four projections run as 3-term fp8e4m3 hi/lo DoubleRow matmuls at 0.75x
the bf16 PE cost with better-than-bf16 accuracy: operands are split into
fp8 hi + unscaled fp8 lo residual (x/W splits on the host for free; the
attention output's split on the DVE), and each psum accumulates
  hi.hi (two 128-chunks per DoubleRow instr) + (lo.hi + hi.lo) (the two
  cross terms as the two k-tiles of one DoubleRow instr), dropping lo.lo.
W's are pre-scaled by WS=64 (descale folded into the exp scale / the final
copy); the attention-output split is kept at 16*o_norm via an extra ln(4)
in the reciprocal so extremes stay inside e4m3 range. Raw fp8 (un-split)
fails the 2e-2 budget everywhere (proj 6.6e-2, QK 4.5e-2, PV 3.1e-2 even
renormalized), so QK/PV stay bf16 -- both are at their structural PE floor
(QK output-bound at M=128, PV contraction-bound at 128 keys).

Attention per (q-chunk, head-pair), transposed layouts throughout:
  S^T = K Q^T per 128-key block (lhsT=KT, K=64, 2 heads side by side in F)
  P^T = exp(S^T * SCALE/WS^2) on ACT straight from PSUM
  rowsums via ones-augmented V (V'' = [V | 1*64], M=128) during the PV
        matmul -- PSUM rows 64:127 hold the rowsum replicated across
        partitions, so no partition-broadcast is ever needed
  0.25/rowsum = exp(-ln(4*rowsum)) on ACT, deferred 4 chunks so it hides
        inside later exp streams.

Schedule: the exp stream (1 elem/partition/cycle on ACT) is ~22% slower
than the QK+PV PE work per key block, so projections are queued as
single-instruction steps and fed into the kb loop between the QK prefetch
and the PV, keeping the PE busy while exp cooks ("pend_hard" = next
stream's projections, deadline-spread; "pend_soft" = out-projections,
fed when slack). Loops run qc-outer so each chunk's out-projection (and
its 2MB output DMA) overlaps the next chunk's attention. Input DMAs are
contiguous-per-partition groups (128 descriptors each), priority-ordered
so the 1.5MB needed by qk_proj(0,0) lands first.

Pitfalls encoded here: GPSIMD cannot touch PSUM (all psum drains go via
nc.any to ACT/DVE); DVE has no float divide; fp8 stores do not saturate
(overflow => inf => NaN); multi-wait instructions are re-legalized into
single-wait NoOp chains for this container's walrus.
"""

import sys

import numpy as np

if "/opt/trn_rl_repo" not in sys.path:
    sys.path.insert(0, "/opt/trn_rl_repo")

import ml_dtypes

B, N, D, H, HD = 4, 2048, 1024, 16, 64
SCALE = HD ** -0.5
NCORES = 8
HPC = H // 2            # heads per core
PAIRS = HPC // 2        # head pairs per core
NKB = N // 128          # key blocks
NQC = N // 512          # query chunks
DC = D // 128           # contraction chunks over D
BF16 = ml_dtypes.bfloat16
E4M3 = ml_dtypes.float8_e4m3
WS = 64.0               # host pre-scale on Wq/Wk/Wv for the fp8 hi/lo split
ESCALE = SCALE / (WS * WS)  # exp() input scale undoing the WS^2 in q.k

_CACHE = {}


def _legalize_bir_waits(bir: bytes) -> bytes:
    """walrus in this container accepts at most ONE sync wait (and update)
    per instruction; Tile emits several. Split excess waits onto preceding
    same-engine NoOps (engines execute their stream in order, so a chain of
    single-wait NoOps is equivalent to one multi-wait instruction), and
    excess updates onto following same-engine NoOps."""
    import orjson

    m = orjson.loads(bir)
    ctr = 0
    for fn in m["functions"]:
        for bb in fn.get("blocks") or []:
            insts = bb.get("instructions")
            if not insts:
                continue
            out = []
            changed = False
            for inst in insts:
                si = inst.get("sync_info")
                eng = inst.get("engine")
                ow = (si or {}).get("on_wait") or []
                if len(ow) > 1 and eng and eng != "Unassigned":
                    for w in ow[:-1]:
                        ctr += 1
                        out.append(
                            {
                                "debug": inst.get("debug", 0),
                                "engine": eng,
                                "ins": [],
                                "name": f"{inst['name']}-lw{ctr}",
                                "opcode": "NoOp",
                                "outs": [],
                                "sync_info": {"on_update": [], "on_wait": [w]},
                            }
                        )
                    si["on_wait"] = [ow[-1]]
                    changed = True
                out.append(inst)
                ou = (si or {}).get("on_update") or []
                if len(ou) > 1 and eng and eng != "Unassigned":
                    for u in ou[1:]:
                        ctr += 1
                        out.append(
                            {
                                "debug": inst.get("debug", 0),
                                "engine": eng,
                                "ins": [],
                                "name": f"{inst['name']}-lu{ctr}",
                                "opcode": "NoOp",
                                "outs": [],
                                "sync_info": {"on_update": [u], "on_wait": []},
                            }
                        )
                    si["on_update"] = [ou[0]]
                    changed = True
            if changed:
                bb["instructions"] = out
    return orjson.dumps(m)


def _install_drain_patch():
    """Route every module serialization through the wait legalizer."""
    if _CACHE.get("drain_patched"):
        return
    import concourse.bass as bass

    orig = bass.Bass.to_json_bytes

    def patched(self):
        return _legalize_bir_waits(orig(self))

    bass.Bass.to_json_bytes = patched
    _CACHE["drain_patched"] = True


def _build_module():
    """Build the (single-NEFF, SPMD) Bass module for one core's work."""
    if "nc" in _CACHE:
        return _CACHE["nc"]
    _install_drain_patch()
    import concourse.bass as bass
    import concourse.mybir as mybir
    import concourse.tile as tile

    bf = mybir.dt.bfloat16
    f8 = mybir.dt.float8e4
    f32 = mybir.dt.float32
    EXP = mybir.ActivationFunctionType.Exp
    LN = mybir.ActivationFunctionType.Ln
    DR = mybir.MatmulPerfMode.DoubleRow

    nc = bass.Bass()
    # x split host-side into fp8 (lo, hi) grouped by 512-query column group;
    # wq/wk split into (hi, lo) grouped by head-pair block; wv by chunk.
    # Each group is contiguous per partition = one cheap DMA.
    xhl = nc.dram_tensor("xhl", (128, NQC, DC, 2, 512), f8, kind="ExternalInput")
    wqhl = nc.dram_tensor("wqhl", (128, PAIRS, DC, 2, 128), f8, kind="ExternalInput")
    wkhl = nc.dram_tensor("wkhl", (128, PAIRS, DC, 2, 128), f8, kind="ExternalInput")
    wvhl = nc.dram_tensor("wvhl", (128, DC, 2, 512), f8, kind="ExternalInput")
    wohl = nc.dram_tensor("wohl", (128, PAIRS, 2, D), f8, kind="ExternalInput")
    cmask = nc.dram_tensor("cmask", (128, 128), bf, kind="ExternalInput")
    f16 = mybir.dt.float16
    # fp16 output: halves the 8MB output DMA (incl. the serialized tail
    # DMAs); 10 mantissa bits add ~5e-4 rel err, host upcasts + sums in f32
    outT = nc.dram_tensor("outT", (D, N), f16, kind="ExternalOutput")

    with tile.TileContext(nc) as tc:
        with (
            tc.tile_pool(name="const", bufs=1) as const,
            tc.tile_pool(name="work", bufs=3) as work,
            tc.tile_pool(name="ps", bufs=2, space="PSUM") as ps,
        ):
            # --- resident SBUF tensors ---------------------------------
            x_sb = const.tile([128, NQC, DC, 2, 512], f8, tag="x_sb", name="x_sb")
            wq_sb = const.tile([128, PAIRS, DC, 2, 128], f8, tag="wq_sb", name="wq_sb")
            wk_sb = const.tile([128, PAIRS, DC, 2, 128], f8, tag="wk_sb", name="wk_sb")
            wv_sb = const.tile([128, DC, 2, 512], f8, tag="wv_sb", name="wv_sb")
            wo_sb = const.tile([128, PAIRS, 2, D], f8, tag="wo_sb", name="wo_sb")
            qt_sb = const.tile([128, PAIRS, N], bf, tag="qt_sb", name="qt_sb")
            kt_sb = const.tile([128, PAIRS, N], bf, tag="kt_sb", name="kt_sb")
            # V augmented with 64 ones-columns: PV matmul output rows 64:127
            # all hold the P^T rowsum, physically replicated across partitions
            v_sb = const.tile([128, NKB, HPC, 128], bf, tag="v_sb", name="v_sb")
            o_sb = const.tile([128, PAIRS, N], bf, tag="o_sb", name="o_sb")
            # fp8 (lo, hi) split of o for the 3-term fp8 out-projection
            ohl_sb = const.tile([128, PAIRS, 2, N], f8, tag="ohl_sb", name="ohl_sb")
            mk_sb = const.tile([128, 128], bf, tag="mk_sb", name="mk_sb")

            # --- input DMAs, priority-ordered for the startup pipeline:
            # qk_proj(0, qc0) needs x group 0 + wq/wk mblk 0 (1.5MB), then
            # V(0..3) adds wv (1MB); later x groups / w blocks stream in
            # ahead of the attention chunks that consume them. Each DMA is
            # one contiguous per-partition run (cheap on HWDGE). ---------
            nc.sync.dma_start(out=wq_sb[:, 0], in_=wqhl[:, 0])
            for jq in range(0, DC, 2):
                nc.sync.dma_start(out=x_sb[:, 0, jq : jq + 2], in_=xhl[:, 0, jq : jq + 2])
            nc.sync.dma_start(out=wk_sb[:, 0], in_=wkhl[:, 0])
            nc.sync.dma_start(out=mk_sb, in_=cmask[:, :])
            nc.sync.dma_start(out=wv_sb[:, 0:4], in_=wvhl[:, 0:4, :, :])
            nc.sync.dma_start(out=wv_sb[:, 4:8], in_=wvhl[:, 4:8, :, :])
            for g in range(1, NQC):
                nc.sync.dma_start(out=x_sb[:, g], in_=xhl[:, g])
                nc.sync.dma_start(out=wq_sb[:, g], in_=wqhl[:, g])
                nc.sync.dma_start(out=wk_sb[:, g], in_=wkhl[:, g])
            for j in range(PAIRS):
                nc.sync.dma_start(out=wo_sb[:, j], in_=wohl[:, j])

            # Projections are emitted as lists of single-instruction steps
            # so they can either run as a burst (run_steps) or be spread
            # one instruction at a time through the attention kb loop,
            # filling the PE while the ACT engine works through the exp
            # stream (which is ~20% slower than QK+PV per key block).
            pend_hard = []   # projection steps needed by the next chunk
            pend_soft = []   # out-projection steps (loose deadline)

            def run_steps(steps):
                for s in steps:
                    s()

            def feed_hard(n):
                for _ in range(min(n, len(pend_hard))):
                    pend_hard.pop(0)()

            def feed_soft(n):
                for _ in range(min(n, len(pend_soft))):
                    pend_soft.pop(0)()

            # 3-term fp8 hi/lo projection: per chunk pair, one DoubleRow
            # matmul for hi*hi (2 k-tiles) plus one DoubleRow per chunk
            # pairing (lo, hi) x-tiles against (hi, lo) w-tiles for the two
            # cross terms. Step order follows chunk DMA arrival.
            def proj3_steps(lhs_of, rhs_of, drain, name):
                st = {}

                def pp():
                    if "t" not in st:
                        st["t"] = ps.tile([128, 512], f32, tag="proj", name=name, bufs=2)
                    return st["t"]

                def mm(j, main, start, stop):
                    def go():
                        nc.tensor.matmul(
                            pp(), lhsT=lhs_of(j, main), rhs=rhs_of(j, main),
                            start=start, stop=stop, perf_mode=DR,
                        )
                    return go

                steps = []
                first = True
                for j in range(0, DC, 2):
                    steps.append(mm(j, None, first, False))
                    first = False
                    steps.append(mm(j, 2, False, False))
                    steps.append(mm(j + 1, None, False, j + 1 == DC - 1))
                steps.append(lambda: drain(st["t"]))
                return steps

            # --- V projection for one 128-row seq block ----------------
            def v_proj_steps(sblk):
                cg, sc = divmod(sblk, 4)
                scol = slice(sc * 128, (sc + 1) * 128)

                def lhs_of(j, main):
                    if main:  # hi tiles of chunks j, j+1
                        return x_sb[:, cg, j : j + 2, 1, scol]
                    return x_sb[:, cg, j, :, scol]  # (lo, hi)

                def rhs_of(j, main):
                    if main:
                        return wv_sb[:, j : j + 2, 0, :]
                    return wv_sb[:, j, :, :]  # (hi, lo)

                def drain(vp):
                    nc.any.tensor_copy(
                        out=v_sb[:, sblk, :, 0:HD],
                        in_=vp.rearrange("p (h d) -> p h d", h=HPC),
                    )

                return proj3_steps(lhs_of, rhs_of, drain, "vp_ps")

            # --- Q^T / K^T projection for one (pair block, q chunk) ----
            def qk_proj_steps(mblk, qc):
                out = []
                for w_sb, dst in ((wq_sb, qt_sb), (wk_sb, kt_sb)):

                    def lhs_of(j, main, w_sb=w_sb):
                        if main:
                            return w_sb[:, mblk, j : j + 2, 0, :]
                        return w_sb[:, mblk, j, :, :]  # (hi, lo)

                    def rhs_of(j, main):
                        if main:
                            return x_sb[:, qc, j : j + 2, 1, :]
                        return x_sb[:, qc, j, :, :]  # (lo, hi)

                    def drain(pp, dst=dst):
                        nc.any.tensor_copy(
                            out=dst[:, mblk, qc * 512 : (qc + 1) * 512],
                            in_=pp,
                        )

                    out.extend(proj3_steps(lhs_of, rhs_of, drain, "qkproj_ps"))
                return out

            def next_stream_steps(a, qc):
                """Projections consumed by the stream after (a, qc)."""
                steps = []
                if a + 1 < PAIRS:
                    steps.extend(qk_proj_steps(a + 1, qc))
                elif qc + 1 < NQC:
                    for sblk in range(4 * (qc + 1), 4 * (qc + 1) + 4):
                        steps.extend(v_proj_steps(sblk))
                    steps.extend(qk_proj_steps(0, qc + 1))
                return steps

            # --- output projection for one q chunk (3-term fp8) --------
            # psum = (64 o_norm)*(64 Wo) = 4096*(o_norm . Wo); the 1/4096
            # descale folds into the drain copy.
            def out_proj_steps(qc, final=False):
                steps = []
                qs = slice(qc * 512, (qc + 1) * 512)
                if final:
                    # last chunk: the in-order PE blocks at the first matmul
                    # needing the final pair's o split, so open all 8 psum
                    # banks (proj+pv+qk rings are idle now), emit every
                    # a0-a2-dependent matmul first, and only then the
                    # a3-dependent ones + drains
                    st = {}

                    def op(ob):
                        if "t" not in st:
                            t = []
                            for _ in range(2):
                                t.append(ps.tile([128, 512], f32, tag="proj", name="opf", bufs=2))
                            for _ in range(2):
                                t.append(ps.tile([128, 512], f32, tag="pv", name="opf2", bufs=2))
                            for _ in range(2):
                                big = ps.tile([128, 1024], f32, tag="qk", name="opf3")
                                t.append(big[:, 0:512])
                                t.append(big[:, 512:1024])
                            st["t"] = t
                        return st["t"][ob]

                    def mm_main_f(ob, a, start):
                        def go():
                            nc.tensor.matmul(
                                op(ob),
                                lhsT=wo_sb[:, a : a + 2, 0, ob * 128 : (ob + 1) * 128],
                                rhs=ohl_sb[:, a : a + 2, 1, qs],
                                start=start, stop=False, perf_mode=DR,
                            )
                        return go

                    def mm_cross_f(ob, a, stop):
                        def go():
                            nc.tensor.matmul(
                                op(ob),
                                lhsT=wo_sb[:, a, :, ob * 128 : (ob + 1) * 128],
                                rhs=ohl_sb[:, a, :, qs],
                                start=False, stop=stop, perf_mode=DR,
                            )
                        return go

                    def drain_f(ob):
                        def go():
                            oc = work.tile([128, 512], f16, tag="oc", name="oc", bufs=5)
                            nc.any.tensor_scalar_mul(oc, op(ob), 1.0 / 1024.0)
                            nc.sync.dma_start(
                                out=outT[ob * 128 : (ob + 1) * 128, qs],
                                in_=oc,
                            )
                        return go

                    for ob in range(8):
                        steps.append(mm_main_f(ob, 0, True))
                        steps.append(mm_cross_f(ob, 0, False))
                        steps.append(mm_cross_f(ob, 1, False))
                        steps.append(mm_cross_f(ob, 2, False))
                    for ob in range(8):
                        steps.append(mm_main_f(ob, 2, False))
                        steps.append(mm_cross_f(ob, 3, True))
                        steps.append(drain_f(ob))
                    return steps
                for ob in range(8):
                    st = {}
                    obs = slice(ob * 128, (ob + 1) * 128)

                    def op(st=st):
                        if "t" not in st:
                            st["t"] = ps.tile([128, 512], f32, tag="proj", name="op_ps", bufs=2)
                        return st["t"]

                    def mm_main(a, start, obs=obs, op=op):
                        def go():
                            nc.tensor.matmul(
                                op(),
                                lhsT=wo_sb[:, a : a + 2, 0, obs],
                                rhs=ohl_sb[:, a : a + 2, 1, qs],
                                start=start, stop=False, perf_mode=DR,
                            )
                        return go

                    def mm_cross(a, stop, obs=obs, op=op):
                        def go():
                            nc.tensor.matmul(
                                op(),
                                lhsT=wo_sb[:, a, :, obs],
                                rhs=ohl_sb[:, a, :, qs],
                                start=False, stop=stop, perf_mode=DR,
                            )
                        return go

                    # a3-dependent matmuls (main2 pairs a2+a3, cross3) go
                    # last so the in-order PE can pre-run the rest while the
                    # final pair's norm/extraction completes
                    steps.append(mm_main(0, True))
                    steps.append(mm_cross(0, False))
                    steps.append(mm_cross(1, False))
                    steps.append(mm_cross(2, False))
                    steps.append(mm_main(2, False))
                    steps.append(mm_cross(3, True))

                    def drain(ob=ob, op=op):
                        oc = work.tile([128, 512], f16, tag="oc", name="oc", bufs=5)
                        nc.any.tensor_scalar_mul(oc, op(), 1.0 / 1024.0)
                        nc.sync.dma_start(
                            out=outT[ob * 128 : (ob + 1) * 128, qc * 512 : (qc + 1) * 512],
                            in_=oc,
                        )

                    steps.append(drain)
                return steps

            # --- attention --------------------------------------------
            # diagonal blocks kb = 4*qc + r only need queries q >= 128*r of
            # the 512-wide chunk (the rest is fully causal-masked): slice
            # QK/exp/PV to q in [128*r, 512) and apply a single 128x128
            # tril mask to the [128r, 128r+128) square.
            def emit_qk(a, qc, kb):
                r = kb - 4 * qc if kb >= 4 * qc else 0
                off = 128 * r
                qk = ps.tile([128, 1024], f32, tag="qk", name="qk_ps")
                for h in range(2):
                    nc.tensor.matmul(
                        qk[:, h * 512 + off : (h + 1) * 512],
                        lhsT=kt_sb[h * 64 : (h + 1) * 64, a, kb * 128 : (kb + 1) * 128],
                        rhs=qt_sb[h * 64 : (h + 1) * 64, a, qc * 512 + off : (qc + 1) * 512],
                        start=True,
                        stop=True,
                    )
                return qk

            norm_q = []

            def emit_norm(a, qc, ou, split=False):
                # ln(4r): the extra ln4 makes rec = 0.25/r, keeping the
                # fp8 o split (16*o_norm, extremes ~75) inside e4m3 range
                rec = work.tile([64, 1024], mybir.dt.float32, tag="rec", name="rec", bufs=2)
                qs = slice(qc * 512, (qc + 1) * 512)
                if not split:
                    nc.scalar.activation(out=rec, in_=ou[64:128, :], func=LN, scale=4.0)
                    nc.scalar.activation(out=rec, in_=rec, func=EXP, scale=-1.0)
                    for h in range(2):
                        nc.vector.tensor_tensor(
                            o_sb[h * 64 : (h + 1) * 64, a, qc * 512 : (qc + 1) * 512],
                            ou[0:HD, h * 512 : (h + 1) * 512],
                            rec[:, h * 512 : (h + 1) * 512],
                            mybir.AluOpType.mult,
                        )
                    nc.vector.tensor_copy(out=ohl_sb[:, a, 1, qs], in_=o_sb[:, a, qs])
                    nc.vector.tensor_sub(
                        ohl_sb[:, a, 0, qs], o_sb[:, a, qs], ohl_sb[:, a, 1, qs]
                    )
                    return
                # per-head pipeline for the final chunk: halves the tail's
                # serialized norm->extract chain by overlapping ACT and DVE
                for h in range(2):
                    hs = slice(h * 512, (h + 1) * 512)
                    hp = slice(h * 64, (h + 1) * 64)
                    hq = slice(qc * 512 + 0, (qc + 1) * 512)
                    nc.scalar.activation(out=rec[:, hs], in_=ou[64:128, hs], func=LN, scale=4.0)
                    nc.scalar.activation(out=rec[:, hs], in_=rec[:, hs], func=EXP, scale=-1.0)
                    nc.vector.tensor_tensor(
                        o_sb[hp, a, hq], ou[0:HD, hs], rec[:, hs], mybir.AluOpType.mult,
                    )
                    nc.vector.tensor_copy(out=ohl_sb[hp, a, 1, qs], in_=o_sb[hp, a, qs])
                    nc.vector.tensor_sub(
                        ohl_sb[hp, a, 0, qs], o_sb[hp, a, qs], ohl_sb[hp, a, 1, qs]
                    )

            # startup: pair-0 qc-0 prereqs as a burst (DMA-paced anyway)
            run_steps(qk_proj_steps(0, 0))
            # ones columns for the augmented-V rowsum trick (split across
            # Pool and DVE, emitted after the first proj so the early DVE
            # drains aren't queued behind the memset)
            nc.gpsimd.memset(v_sb[:, 0 : NKB // 2, :, HD:128], 1.0)
            nc.vector.memset(v_sb[:, NKB // 2 : NKB, :, HD:128], 1.0)
            for sblk in range(4):
                run_steps(v_proj_steps(sblk))
            # qc-outer / pair-inner: out-projection of chunk qc (and its
            # 2MB of output DMA) runs during chunk qc+1's attention, so the
            # output stream spreads across the whole run instead of piling
            # up behind the last pair.
            for qc in range(NQC):
                nkb = 4 * qc + 4
                for a in range(PAIRS):
                    # from qc>=1 the next stream's projections interleave
                    # into this chunk's kb loop (hard deadline: next chunk);
                    # during qc==0 the input DMAs are still streaming, so
                    # they burst at chunk end instead (legacy behavior)
                    if qc >= 1:
                        pend_hard.extend(next_stream_steps(a, qc))
                    # pv psum per head: half-sized tiles drain (and free)
                    # independently, halving the next chunk's PV wait
                    pvh = [
                        ps.tile([128, 512], f32, tag="pv", name="pv_ps", bufs=2)
                        for _ in range(2)
                    ]
                    qk_q = [emit_qk(a, qc, kb) for kb in range(min(2, nkb))]
                    for kb in range(nkb):
                        qk = qk_q.pop(0)
                        if kb + 2 < nkb:
                            qk_q.append(emit_qk(a, qc, kb + 2))
                        rem = max(1, nkb - 1 - kb)
                        feed_hard(-(-len(pend_hard) // rem))
                        if qc == NQC - 1 and a == PAIRS - 1:
                            feed_soft(2)
                        elif not pend_hard and len(pend_soft) > 8:
                            feed_soft(2)
                        r = kb - 4 * qc if kb >= 4 * qc else 0
                        off = 128 * r
                        pt = work.tile([128, 2, 512], bf, tag="pt", name="pt", bufs=4)
                        if r == 0:
                            nc.scalar.activation(
                                out=pt.rearrange("p h q -> p (h q)"),
                                in_=qk[:, :],
                                func=EXP,
                                scale=ESCALE,
                            )
                        else:
                            nc.scalar.activation(
                                out=pt[:, :, off:512],
                                in_=qk.rearrange("p (h q) -> p h q", h=2)[:, :, off:512],
                                func=EXP,
                                scale=ESCALE,
                            )
                        if kb >= 4 * qc:
                            nc.vector.tensor_mul(
                                pt[:, :, off : off + 128],
                                pt[:, :, off : off + 128],
                                mk_sb[:, None, :].to_broadcast([128, 2, 128]),
                            )
                        for h in range(2):
                            nc.tensor.matmul(
                                pvh[h][:, off:512],
                                lhsT=v_sb[:, kb, 2 * a + h, :],
                                rhs=pt[:, h, off:512],
                                start=(kb == 0),
                                stop=(kb == nkb - 1),
                                skip_group_check=True,
                            )
                    # copy PV psum to SBUF right away (frees the pv slots),
                    # but defer the normalization (ln/exp reciprocal +
                    # multiply) so the ACT reciprocal hides inside later
                    # exp streams
                    feed_hard(len(pend_hard))  # safety: next chunk needs these
                    ou = work.tile([128, 1024], mybir.dt.float32, tag="ou", name="ou", bufs=5)
                    for h in range(2):
                        nc.vector.tensor_copy(out=ou[:, h * 512 : (h + 1) * 512], in_=pvh[h])
                    norm_q.append((a, qc, ou))
                    if qc == 0:
                        # burst next stream's projections (see above)
                        run_steps(next_stream_steps(a, qc))
                    # drain the norm queue gradually through the last chunk
                    # so the final out-proj isn't gated on a norm cascade
                    # after the last PV
                    lag = (3 - a) if qc == NQC - 1 else 4
                    while len(norm_q) > lag:
                        na, nqc, nou = norm_q.pop(0)
                        emit_norm(na, nqc, nou)
                        if na == PAIRS - 1:
                            pend_soft.extend(out_proj_steps(nqc))
            while norm_q:
                na, nqc, nou = norm_q.pop(0)
                emit_norm(na, nqc, nou, split=not norm_q)
                if na == PAIRS - 1:
                    pend_soft.extend(out_proj_steps(nqc))
            feed_soft(len(pend_soft))

    _CACHE["nc"] = nc
    return nc


def _causal_masks():
    k = np.arange(128)[:, None]
    q = np.arange(128)[None, :]
    return (q >= k).astype(BF16)


def _split_pair(a, lo_first):
    """[D, M] f32 -> (lo/hi-stacked [2, DC, 128, M] fp8)."""
    ar = np.ascontiguousarray(a).reshape(DC, 128, -1)
    hi = ar.astype(E4M3)
    lo = (ar - hi.astype(np.float32)).astype(E4M3)
    pair = (lo, hi) if lo_first else (hi, lo)
    return np.stack(pair, axis=0)


def _split_x(a):
    """x.T [D, N] -> [128, NQC, DC, 2, 512] fp8 (lo, hi) by column group."""
    s = _split_pair(a, lo_first=True)  # [2, DC, 128, N]
    s = s.reshape(2, DC, 128, NQC, 512)
    return np.ascontiguousarray(s.transpose(2, 3, 1, 0, 4))


def _split_w(a):
    """W.T [D, 512] -> [128, PAIRS, DC, 2, 128] fp8 (hi, lo) by pair block."""
    s = _split_pair(a, lo_first=False)  # [2, DC, 128, 512]
    s = s.reshape(2, DC, 128, PAIRS, 128)
    return np.ascontiguousarray(s.transpose(2, 3, 1, 0, 4))


def _split_wv(a):
    """Wv.T [D, 512] -> [128, DC, 2, 512] fp8 (hi, lo) by chunk."""
    s = _split_pair(a, lo_first=False)  # [2, DC, 128, 512]
    return np.ascontiguousarray(s.transpose(2, 1, 0, 3))


def _split_wo(a):
    """Wo.T slice [512, D] -> [128, PAIRS, 2, D] fp8 (hi, lo) by pair."""
    ar = np.ascontiguousarray(a).reshape(PAIRS, 128, D)
    hi = ar.astype(E4M3)
    lo = (ar - hi.astype(np.float32)).astype(E4M3)
    s = np.stack((hi, lo), axis=0)  # [2, PAIRS, 128, D]
    return np.ascontiguousarray(s.transpose(2, 1, 0, 3))


def _prep_in_maps(x, Wq, Wk, Wv, Wo):
    cm = _causal_masks()
    in_maps = []
    xhl_b = [_split_x(x[b].T) for b in range(B)]
    for c in range(NCORES):
        b, hg = divmod(c, 2)
        rs = slice(hg * 512, (hg + 1) * 512)
        in_maps.append(
            {
                "xhl": xhl_b[b],
                "wqhl": _split_w(Wq[rs].T * WS),
                "wkhl": _split_w(Wk[rs].T * WS),
                "wvhl": _split_wv(Wv[rs].T * WS),
                "wohl": _split_wo(Wo[:, rs].T * WS),
                "cmask": cm,
            }
        )
    return in_maps


def _is_causal(mask):
    mask = np.asarray(mask)
    if mask.shape != (N, N):
        return False
    return bool(np.array_equal(mask, np.tril(np.ones((N, N), dtype=bool))))


def _numpy_fallback(x, mask, Wq, Wk, Wv, Wo):
    out = np.empty((B, N, D), np.float32)
    madd = np.where(np.asarray(mask), 0.0, -np.inf).astype(np.float32)
    for b in range(B):
        q = (x[b] @ Wq.T).reshape(N, H, HD).transpose(1, 0, 2)
        k = (x[b] @ Wk.T).reshape(N, H, HD).transpose(1, 0, 2)
        v = (x[b] @ Wv.T).reshape(N, H, HD).transpose(1, 0, 2)
        o = np.empty((H, N, HD), np.float32)
        for h in range(H):
            s = q[h] @ k[h].T * SCALE + madd
            s -= s.max(axis=-1, keepdims=True)
            p = np.exp(s)
            p /= p.sum(axis=-1, keepdims=True)
            o[h] = p @ v[h]
        out[b] = o.transpose(1, 0, 2).reshape(N, D) @ Wo.T
    return out


def _run_device(x, Wq, Wk, Wv, Wo):
    from concourse.bass_utils import run_bass_kernel_spmd

    nc = _build_module()
    in_maps = _prep_in_maps(x, Wq, Wk, Wv, Wo)
    res = run_bass_kernel_spmd(nc, in_maps, core_ids=list(range(NCORES)))
    outs = [np.asarray(r["outT"], dtype=np.float32) for r in res.results]
    out = np.empty((B, N, D), np.float32)
    for b in range(B):
        out[b] = (outs[2 * b] + outs[2 * b + 1]).T
    return out


def kernel(x, mask, Wq, Wk, Wv, Wo):
    x = np.asarray(x, dtype=np.float32)
    Wq = np.asarray(Wq, dtype=np.float32)
    Wk = np.asarray(Wk, dtype=np.float32)
    Wv = np.asarray(Wv, dtype=np.float32)
    Wo = np.asarray(Wo, dtype=np.float32)
    if not _is_causal(mask):
        return _numpy_fallback(x, mask, Wq, Wk, Wv, Wo)
    try:
        return _run_device(x, Wq, Wk, Wv, Wo)
    except Exception:
        try:
            return _run_device(x, Wq, Wk, Wv, Wo)
        except Exception:
            # last resort: slow but correct
            return _numpy_fallback(x, mask, Wq, Wk, Wv, Wo)


def simulate():
    """Cost-model timeline estimate of one core's NEFF execution (ns)."""
    from concourse.timeline_sim import TimelineSim

    nc = _build_module()
    return TimelineSim(nc).simulate()

